# revision 9
# baseline (speedup 1.0000x reference)
"""VQ codebook nearest-neighbour kernel for 8 trn2 NeuronCores.

Problem: z_e (16, 64, 12000) f32, weight (512, 64) f32
         -> indices (16, 12000) int32 = argmin_k ||z[:, t] - w[k]||^2

Per core (2 batches, data parallel, no collectives), per 125-token tile:
  - PE: two full-rate 16-bit matmuls accumulate
        psum[t, j] = [z1; z2]_f16 . [w1; w1]_f16 + [zb1; zb2]_bf16 . [wlo; wlo]_bf16
    where z = z1 + z2 is an fp16 split (exact to ~2^-22), w1 = fp16(w),
    wlo = bf16(w - w1) (bf16 because w - w1 is fp16-subnormal and the PE
    flushes fp16 denormals), zb = bf16 2-term split of z. HW-measured to
    reproduce the fp32 matmul to 3e-8 (the f32 matmul's own accumulation
    noise, far below the reference's f32 rounding grid) at 1 cycle/column
    instead of fp32's 4. Columns host-reversed (k = 511 - j).
  - ACT: dist = relu(psum + z_sq[t])  (exact: dist > 0), per-token bias,
    evacuating PSUM -> SBUF with the same f32 rounding as the reference's
    (z_sq - 2 cross).
  - DVE: one custom instruction per tile: d = Src0 + Src1 (+e_sq[k], the
    reference's final rounding), running-min scan, select(Idx), max-accum.
    On the reversed column order the accum returns 511 - (first argmin).
  - Epilogue: k = 511 - acc, PE-transpose per batch, int32 cast, DMA out.
"""

import ml_dtypes
import numpy as np

import concourse.bacc as bacc
import concourse.bass as bass
import concourse.mybir as mybir
import concourse.tile as tile
from concourse import dve_ops, masks
from concourse.bass_utils import run_bass_kernel_spmd
from concourse.dve_spec import (
    AluOp,
    Bin,
    C0,
    Idx,
    MaxNeg,
    Spec,
    Src0,
    Src1,
    Zero,
    lower,
    maxx,
    scan,
    select,
)
from concourse.dve_uop import DveOpSpec

# ---------------------------------------------------------------- constants
B, D, T, K = 16, 64, 12000, 512
NCORES = 8
BPC = B // NCORES          # batches per core = 2
NTOK = BPC * T             # tokens per core = 24000
TILE = 125                 # tokens per matmul tile (96 tiles per batch)
TPB = T // TILE            # tiles per batch = 96
NTILES = BPC * TPB         # 192
CHUNK = 2000               # tokens per input DMA (16 tiles)
TPC = CHUNK // TILE        # tiles per chunk = 16
NCHUNK = T // CHUNK        # chunks per batch = 6
DT_F32 = mybir.dt.float32
DT_BF16 = mybir.dt.bfloat16
DT_F16 = mybir.dt.float16
DT_I32 = mybir.dt.int32
BF16 = ml_dtypes.bfloat16

# ------------------------------------------------- custom DVE argmin op


def _argmin_ref(in0, in1, c0, c1, c2):
    """CoreSim reference. d = in0 + in1 (f32); body = where(d == runmin(d),
    idx, -FLT_MAX); accum = max over free axis."""
    p = in0.shape[0]
    x = np.asarray(in0, np.float32).reshape(p, -1)
    x = (x + np.asarray(in1, np.float32).reshape(p, -1)).astype(np.float32)
    m = np.minimum.accumulate(x, axis=1)
    idx = np.arange(x.shape[1], dtype=np.float32)[None, :]
    body = np.where(x <= m, idx, np.float32(-np.finfo(np.float32).max))
    acc = body.max(axis=-1, keepdims=True)
    return body.reshape(in0.shape), acc


def _argmin_ref_c0(in0, in1, c0, c1, c2):
    """Like _argmin_ref but d = (in0 + c0) + in1 (z_sq via per-partition C0)."""
    p = in0.shape[0]
    x = np.asarray(in0, np.float32).reshape(p, -1)
    x = (x + np.asarray(c0, np.float32).reshape(p, 1)).astype(np.float32)
    x = (x + np.asarray(in1, np.float32).reshape(p, -1)).astype(np.float32)
    m = np.minimum.accumulate(x, axis=1)
    idx = np.arange(x.shape[1], dtype=np.float32)[None, :]
    body = np.where(x <= m, idx, np.float32(-np.finfo(np.float32).max))
    acc = body.max(axis=-1, keepdims=True)
    return body.reshape(in0.shape), acc


def _mk_argmin_spec(with_c0):
    d = (Src0 + C0) + Src1 if with_c0 else Src0 + Src1
    runmin = scan(AluOp.MIN, d, init=Bin(AluOp.SUBTRACT, Zero, MaxNeg))
    return Spec(
        body=select(d <= runmin, Idx, MaxNeg),
        accum=maxx,
        accum_init=MaxNeg,
        reference=_argmin_ref_c0 if with_c0 else _argmin_ref,
    )


def _register(name, spec):
    for op in dve_ops.OPS:
        if op.name == name:
            return op
    opcode = dve_ops._CUSTOM_DVE_ROW_BASE + len(dve_ops.OPS)
    shas = {}
    for ver in ("v3", "v4"):
        s = DveOpSpec(name=name, opcode=opcode, uops=lower(spec, ver=ver), rd1_en=True)
        shas[ver] = s.sha(ver)
    op = dve_ops.DveOp(name, spec, subdim=False, uops_sha=shas)
    dve_ops.OPS.append(op)
    dve_ops.CUSTOM_DVE_SPECS[name] = spec
    dve_ops._SUB_OPCODE_FOR_NAME[name] = opcode
    return op


ARGMIN_OP = _register("ARGMIN_FIRST2_ANT", _mk_argmin_spec(False))
ARGMIN_C0_OP = _register("ARGMIN_FIRSTC0_ANT", _mk_argmin_spec(True))

# ------------------------------------------------------------ device kernel


def build_nc(use_act=True, zpool_bufs=4, psum_bufs=6, dist_bufs=8):
    nc = bacc.Bacc("TRN2", target_bir_lowering=False, debug=False)
    z16 = nc.dram_tensor("z2_16", [2 * D, NTOK], DT_F16, kind="ExternalInput").ap()
    zbf = nc.dram_tensor("z2_bf", [2 * D, NTOK], DT_BF16, kind="ExternalInput").ap()
    wa = nc.dram_tensor("w_hi2", [2 * D, K], DT_F16, kind="ExternalInput").ap()
    wb = nc.dram_tensor("w_lo2", [2 * D, K], DT_BF16, kind="ExternalInput").ap()
    zsq = nc.dram_tensor("zsq_in", [TILE, NTILES], DT_F32, kind="ExternalInput").ap()
    esq = nc.dram_tensor("esq_rev", [TILE, K], DT_F32, kind="ExternalInput").ap()
    out = nc.dram_tensor("idx_out", [NTOK], DT_I32, kind="ExternalOutput").ap()

    relu = mybir.ActivationFunctionType.Relu

    with tile.TileContext(nc) as tc:
        with (
            tc.tile_pool(name="const", bufs=1) as constp,
            tc.tile_pool(name="zchunk", bufs=zpool_bufs) as zpool,
            tc.tile_pool(name="scores", bufs=psum_bufs, space="PSUM") as pspool,
            tc.tile_pool(name="dist", bufs=dist_bufs) as distp,
            tc.tile_pool(name="junk", bufs=1) as junkp,
            tc.tile_pool(name="acc", bufs=1) as accp,
            tc.tile_pool(name="tpose", bufs=2, space="PSUM") as tpp,
            tc.tile_pool(name="outi", bufs=2) as outp,
        ):
            wa_sb = constp.tile([2 * D, K], DT_F16)
            nc.sync.dma_start(wa_sb[:], wa)
            wb_sb = constp.tile([2 * D, K], DT_BF16)
            nc.sync.dma_start(wb_sb[:], wb)
            esq_sb = constp.tile([TILE, K], DT_F32)
            nc.sync.dma_start(esq_sb[:], esq)
            zsq_sb = constp.tile([TILE, NTILES], DT_F32)
            nc.sync.dma_start(zsq_sb[:], zsq)
            ident = constp.tile([TILE, TILE], DT_F32)
            masks.make_identity(nc, ident[:])
            junk = junkp.tile([TILE, K], DT_F32)
            acc = accp.tile([TILE, NTILES], DT_F32)

            for b in range(BPC):
                for c in range(NCHUNK):
                    zt16 = zpool.tile([2 * D, CHUNK], DT_F16, tag="z16")
                    ztbf = zpool.tile([2 * D, CHUNK], DT_BF16, tag="zbf")
                    t0 = (b * NCHUNK + c) * CHUNK
                    nc.sync.dma_start(zt16[:], z16[:, t0 : t0 + CHUNK])
                    nc.sync.dma_start(ztbf[:], zbf[:, t0 : t0 + CHUNK])
                    for i in range(TPC):
                        j = (b * NCHUNK + c) * TPC + i  # global tile id
                        ps = pspool.tile([TILE, K], DT_F32)
                        nc.tensor.matmul(
                            ps[:],
                            lhsT=zt16[:, i * TILE : (i + 1) * TILE],
                            rhs=wa_sb[:],
                            start=True,
                            stop=False,
                        )
                        nc.tensor.matmul(
                            ps[:],
                            lhsT=ztbf[:, i * TILE : (i + 1) * TILE],
                            rhs=wb_sb[:],
                            start=False,
                            stop=True,
                        )
                        if use_act:
                            dist = distp.tile([TILE, K], DT_F32)
                            nc.scalar.activation(
                                dist[:], ps[:], relu,
                                bias=zsq_sb[:, j : j + 1], scale=1.0,
                            )
                            nc.vector._custom_dve(
                                ARGMIN_OP,
                                out=junk[:],
                                in0=dist[:],
                                in1=esq_sb[:],
                                accum_out=acc[:, j : j + 1],
                            )
                        else:
                            nc.vector._custom_dve(
                                ARGMIN_C0_OP,
                                out=junk[:],
                                in0=ps[:],
                                in1=esq_sb[:],
                                s0=zsq_sb[:, j : j + 1],
                                accum_out=acc[:, j : j + 1],
                            )

            # k = 511 - acc  (columns were host-reversed)
            acck = accp.tile([TILE, NTILES], DT_F32)
            nc.vector.tensor_scalar(
                acck[:],
                acc[:],
                -1.0,
                float(K - 1),
                op0=mybir.AluOpType.mult,
                op1=mybir.AluOpType.add,
            )

            for b in range(BPC):
                pt = tpp.tile([TPB, TILE], DT_F32)
                nc.tensor.transpose(
                    pt[:], acck[:, b * TPB : (b + 1) * TPB], ident[:]
                )
                oi = outp.tile([TPB, TILE], DT_I32)
                nc.vector.tensor_copy(oi[:], pt[:])
                dst = out[b * T : (b + 1) * T].rearrange("(j p) -> j p", p=TILE)
                nc.sync.dma_start(dst, oi[:])

    nc.compile()
    return nc


_NC_CACHE = None


def _get_nc():
    global _NC_CACHE
    if _NC_CACHE is None:
        _NC_CACHE = build_nc(**BUILD_KWARGS)
    return _NC_CACHE


BUILD_KWARGS = {}


# -------------------------------------------------------------- host driver

LAST_RESULTS = None  # BassKernelResults of the most recent run (for test.py)


def _host_prep(z_e, w):
    """z_sq / e_sq via jax-CPU to mirror the reference's f32 reductions."""
    import jax
    import jax.numpy as jnp

    cpu = jax.devices("cpu")[0]
    with jax.default_device(cpu):
        zj = jnp.transpose(jnp.asarray(z_e), (0, 2, 1))
        z_sq = np.asarray(jnp.sum(zj * zj, axis=-1))          # (B, T) f32
        wj = jnp.asarray(w)
        e_sq = np.asarray(jnp.sum(wj * wj, axis=-1))          # (K,) f32
    return z_sq, e_sq


def _split_bf16(x):
    hi = x.astype(BF16)
    lo = (x - hi.astype(np.float32)).astype(BF16)
    return hi, lo


def _split_f16(x):
    hi = x.astype(np.float16)
    lo = (x - hi.astype(np.float32)).astype(np.float16)
    return hi, lo


def kernel(z_e, weight, _trace=False):
    z_e = np.ascontiguousarray(np.asarray(z_e, dtype=np.float32))
    w = np.ascontiguousarray(np.asarray(weight, dtype=np.float32))
    assert z_e.shape == (B, D, T) and w.shape == (K, D)

    z_sq, e_sq = _host_prep(z_e, w)

    # moving operands: column j holds code k = 511-j, scaled by -2 (exact)
    wt = np.ascontiguousarray((-2.0 * w.T)[:, ::-1]).astype(np.float32)  # (D, K)
    w_hi = wt.astype(np.float16)
    w_lob = (wt - w_hi.astype(np.float32)).astype(BF16)
    w_hi2 = np.ascontiguousarray(np.concatenate([w_hi, w_hi], axis=0))   # (128, K) f16
    w_lo2 = np.ascontiguousarray(np.concatenate([w_lob, w_lob], axis=0))  # (128, K) bf16
    esq_rev = np.ascontiguousarray(
        np.broadcast_to(e_sq[::-1], (TILE, K))
    ).astype(np.float32)

    in_maps = []
    for c in range(NCORES):
        zc = np.empty((D, NTOK), np.float32)
        for b in range(BPC):
            zc[:, b * T : (b + 1) * T] = z_e[BPC * c + b]
        zh16, zl16 = _split_f16(zc)
        z2_16 = np.empty((2 * D, NTOK), np.float16)
        z2_16[:D] = zh16
        z2_16[D:] = zl16
        zhb, zlb = _split_bf16(zc)
        z2_bf = np.empty((2 * D, NTOK), BF16)
        z2_bf[:D] = zhb
        z2_bf[D:] = zlb
        # zsq laid out [TILE, NTILES]: (p, j) -> token j*TILE + p
        zsqc = np.ascontiguousarray(
            z_sq[BPC * c : BPC * (c + 1)].reshape(NTILES, TILE).T
        ).astype(np.float32)
        in_maps.append(
            {
                "z2_16": z2_16,
                "z2_bf": z2_bf,
                "w_hi2": w_hi2,
                "w_lo2": w_lo2,
                "zsq_in": zsqc,
                "esq_rev": esq_rev,
            }
        )

    nc = _get_nc()
    global LAST_RESULTS
    # transient NRT/axon device hiccups have been observed; retry
    for attempt in range(3):
        try:
            LAST_RESULTS = run_bass_kernel_spmd(
                nc, in_maps, list(range(NCORES)), trace=_trace
            )
            break
        except Exception:
            if attempt == 2:
                raise
            import time as _time

            _time.sleep(2.0 * (attempt + 1))

    out = np.empty((B, T), np.int32)
    for c in range(NCORES):
        o = np.asarray(LAST_RESULTS.results[c]["idx_out"]).reshape(BPC, T)
        for b in range(BPC):
            out[BPC * c + b] = o[b]
    return out


# revision 10
# speedup vs baseline: 1.0054x; 1.0054x over previous
"""VQ codebook nearest-neighbour kernel for 8 trn2 NeuronCores.

Problem: z_e (16, 64, 12000) f32, weight (512, 64) f32
         -> indices (16, 12000) int32 = argmin_k ||z[:, t] - w[k]||^2

Per core (2 batches, data parallel, no collectives), per 125-token tile:
  - PE: two full-rate 16-bit matmuls accumulate
        psum[t, j] = [z1; z2]_f16 . [w1; w1]_f16 + [zb1; zb2]_bf16 . [wlo; wlo]_bf16
    where z = z1 + z2 is an fp16 split (exact to ~2^-22), w1 = fp16(w),
    wlo = bf16(w - w1) (bf16 because w - w1 is fp16-subnormal and the PE
    flushes fp16 denormals), zb = bf16 2-term split of z. HW-measured to
    reproduce the fp32 matmul to 3e-8 (the f32 matmul's own accumulation
    noise, far below the reference's f32 rounding grid) at 1 cycle/column
    instead of fp32's 4. Columns host-reversed (k = 511 - j).
  - ACT: dist = relu(psum + z_sq[t])  (exact: dist > 0), per-token bias,
    evacuating PSUM -> SBUF with the same f32 rounding as the reference's
    (z_sq - 2 cross).
  - DVE: one custom instruction per tile: d = Src0 + Src1 (+e_sq[k], the
    reference's final rounding), running-min scan, select(Idx), max-accum.
    On the reversed column order the accum returns 511 - (first argmin).
  - Epilogue: k = 511 - acc, PE-transpose per batch, int32 cast, DMA out.
"""

import ml_dtypes
import numpy as np

import concourse.bacc as bacc
import concourse.bass as bass
import concourse.mybir as mybir
import concourse.tile as tile
from concourse import dve_ops, masks
from concourse.bass_utils import run_bass_kernel_spmd
from concourse.dve_spec import (
    AluOp,
    Bin,
    C0,
    Idx,
    MaxNeg,
    Spec,
    Src0,
    Src1,
    Zero,
    lower,
    maxx,
    scan,
    select,
)
from concourse.dve_uop import DveOpSpec

# ---------------------------------------------------------------- constants
B, D, T, K = 16, 64, 12000, 512
NCORES = 8
BPC = B // NCORES          # batches per core = 2
NTOK = BPC * T             # tokens per core = 24000
TILE = 125                 # tokens per matmul tile (96 tiles per batch)
TPB = T // TILE            # tiles per batch = 96
NTILES = BPC * TPB         # 192
CHUNK = 2000               # tokens per input DMA (16 tiles)
TPC = CHUNK // TILE        # tiles per chunk = 16
NCHUNK = T // CHUNK        # chunks per batch = 6
DT_F32 = mybir.dt.float32
DT_BF16 = mybir.dt.bfloat16
DT_F16 = mybir.dt.float16
DT_I32 = mybir.dt.int32
BF16 = ml_dtypes.bfloat16

# ------------------------------------------------- custom DVE argmin op


def _argmin_ref(in0, in1, c0, c1, c2):
    """CoreSim reference. d = in0 + in1 (f32); body = where(d == runmin(d),
    idx, -FLT_MAX); accum = max over free axis."""
    p = in0.shape[0]
    x = np.asarray(in0, np.float32).reshape(p, -1)
    x = (x + np.asarray(in1, np.float32).reshape(p, -1)).astype(np.float32)
    m = np.minimum.accumulate(x, axis=1)
    idx = np.arange(x.shape[1], dtype=np.float32)[None, :]
    body = np.where(x <= m, idx, np.float32(-np.finfo(np.float32).max))
    acc = body.max(axis=-1, keepdims=True)
    return body.reshape(in0.shape), acc


def _argmin_ref_c0(in0, in1, c0, c1, c2):
    """Like _argmin_ref but d = (in0 + c0) + in1 (z_sq via per-partition C0)."""
    p = in0.shape[0]
    x = np.asarray(in0, np.float32).reshape(p, -1)
    x = (x + np.asarray(c0, np.float32).reshape(p, 1)).astype(np.float32)
    x = (x + np.asarray(in1, np.float32).reshape(p, -1)).astype(np.float32)
    m = np.minimum.accumulate(x, axis=1)
    idx = np.arange(x.shape[1], dtype=np.float32)[None, :]
    body = np.where(x <= m, idx, np.float32(-np.finfo(np.float32).max))
    acc = body.max(axis=-1, keepdims=True)
    return body.reshape(in0.shape), acc


def _mk_argmin_spec(with_c0):
    d = (Src0 + C0) + Src1 if with_c0 else Src0 + Src1
    runmin = scan(AluOp.MIN, d, init=Bin(AluOp.SUBTRACT, Zero, MaxNeg))
    return Spec(
        body=select(d <= runmin, Idx, MaxNeg),
        accum=maxx,
        accum_init=MaxNeg,
        reference=_argmin_ref_c0 if with_c0 else _argmin_ref,
    )


def _register(name, spec):
    for op in dve_ops.OPS:
        if op.name == name:
            return op
    opcode = dve_ops._CUSTOM_DVE_ROW_BASE + len(dve_ops.OPS)
    shas = {}
    for ver in ("v3", "v4"):
        s = DveOpSpec(name=name, opcode=opcode, uops=lower(spec, ver=ver), rd1_en=True)
        shas[ver] = s.sha(ver)
    op = dve_ops.DveOp(name, spec, subdim=False, uops_sha=shas)
    dve_ops.OPS.append(op)
    dve_ops.CUSTOM_DVE_SPECS[name] = spec
    dve_ops._SUB_OPCODE_FOR_NAME[name] = opcode
    return op


ARGMIN_OP = _register("ARGMIN_FIRST2_ANT", _mk_argmin_spec(False))
ARGMIN_C0_OP = _register("ARGMIN_FIRSTC0_ANT", _mk_argmin_spec(True))

# ------------------------------------------------------------ device kernel


def build_nc(use_act=True, zpool_bufs=4, psum_bufs=6, dist_bufs=8):
    nc = bacc.Bacc("TRN2", target_bir_lowering=False, debug=False)
    z16 = nc.dram_tensor("z2_16", [2 * D, NTOK], DT_F16, kind="ExternalInput").ap()
    zbf = nc.dram_tensor("z2_bf", [2 * D, NTOK], DT_BF16, kind="ExternalInput").ap()
    wa = nc.dram_tensor("w_hi2", [2 * D, K], DT_F16, kind="ExternalInput").ap()
    wb = nc.dram_tensor("w_lo2", [2 * D, K], DT_BF16, kind="ExternalInput").ap()
    zsq = nc.dram_tensor("zsq_in", [TILE, NTILES], DT_F32, kind="ExternalInput").ap()
    esq = nc.dram_tensor("esq_rev", [TILE, K], DT_F32, kind="ExternalInput").ap()
    out = nc.dram_tensor("idx_out", [NTOK], DT_I32, kind="ExternalOutput").ap()

    relu = mybir.ActivationFunctionType.Relu

    with tile.TileContext(nc) as tc:
        with (
            tc.tile_pool(name="const", bufs=1) as constp,
            tc.tile_pool(name="zchunk", bufs=zpool_bufs) as zpool,
            tc.tile_pool(name="scores", bufs=psum_bufs, space="PSUM") as pspool,
            tc.tile_pool(name="dist", bufs=dist_bufs) as distp,
            tc.tile_pool(name="junk", bufs=1) as junkp,
            tc.tile_pool(name="acc", bufs=2) as accp,
            tc.tile_pool(name="tpose", bufs=2, space="PSUM") as tpp,
            tc.tile_pool(name="outi", bufs=2) as outp,
        ):
            wa_sb = constp.tile([2 * D, K], DT_F16)
            nc.sync.dma_start(wa_sb[:], wa)
            wb_sb = constp.tile([2 * D, K], DT_BF16)
            nc.sync.dma_start(wb_sb[:], wb)
            esq_sb = constp.tile([TILE, K], DT_F32)
            nc.sync.dma_start(esq_sb[:], esq)
            zsq_sb = constp.tile([TILE, NTILES], DT_F32)
            nc.sync.dma_start(zsq_sb[:], zsq)
            ident = constp.tile([TILE, TILE], DT_F32)
            masks.make_identity(nc, ident[:])
            junk = junkp.tile([TILE, K], DT_F32)
            acc = accp.tile([TILE, NTILES], DT_F32)

            for b in range(BPC):
                for c in range(NCHUNK):
                    zt16 = zpool.tile([2 * D, CHUNK], DT_F16, tag="z16")
                    ztbf = zpool.tile([2 * D, CHUNK], DT_BF16, tag="zbf")
                    t0 = (b * NCHUNK + c) * CHUNK
                    nc.sync.dma_start(zt16[:], z16[:, t0 : t0 + CHUNK])
                    nc.sync.dma_start(ztbf[:], zbf[:, t0 : t0 + CHUNK])
                    for i in range(TPC):
                        j = (b * NCHUNK + c) * TPC + i  # global tile id
                        ps = pspool.tile([TILE, K], DT_F32)
                        nc.tensor.matmul(
                            ps[:],
                            lhsT=zt16[:, i * TILE : (i + 1) * TILE],
                            rhs=wa_sb[:],
                            start=True,
                            stop=False,
                        )
                        nc.tensor.matmul(
                            ps[:],
                            lhsT=ztbf[:, i * TILE : (i + 1) * TILE],
                            rhs=wb_sb[:],
                            start=False,
                            stop=True,
                        )
                        if use_act:
                            dist = distp.tile([TILE, K], DT_F32)
                            nc.scalar.activation(
                                dist[:], ps[:], relu,
                                bias=zsq_sb[:, j : j + 1], scale=1.0,
                            )
                            nc.vector._custom_dve(
                                ARGMIN_OP,
                                out=junk[:],
                                in0=dist[:],
                                in1=esq_sb[:],
                                accum_out=acc[:, j : j + 1],
                            )
                        else:
                            nc.vector._custom_dve(
                                ARGMIN_C0_OP,
                                out=junk[:],
                                in0=ps[:],
                                in1=esq_sb[:],
                                s0=zsq_sb[:, j : j + 1],
                                accum_out=acc[:, j : j + 1],
                            )

                # batch epilogue overlaps the next batch's compute:
                # k = 511 - acc (columns were host-reversed), transpose,
                # int cast, contiguous DMA out
                acck = accp.tile([TILE, TPB], DT_F32, tag="acck")
                nc.vector.tensor_scalar(
                    acck[:],
                    acc[:, b * TPB : (b + 1) * TPB],
                    -1.0,
                    float(K - 1),
                    op0=mybir.AluOpType.mult,
                    op1=mybir.AluOpType.add,
                )
                pt = tpp.tile([TPB, TILE], DT_F32)
                nc.tensor.transpose(pt[:], acck[:], ident[:])
                oi = outp.tile([TPB, TILE], DT_I32)
                nc.vector.tensor_copy(oi[:], pt[:])
                dst = out[b * T : (b + 1) * T].rearrange("(j p) -> j p", p=TILE)
                nc.sync.dma_start(dst, oi[:])


    nc.compile()
    return nc


_NC_CACHE = None


def _get_nc():
    global _NC_CACHE
    if _NC_CACHE is None:
        _NC_CACHE = build_nc(**BUILD_KWARGS)
    return _NC_CACHE


BUILD_KWARGS = {}


# -------------------------------------------------------------- host driver

LAST_RESULTS = None  # BassKernelResults of the most recent run (for test.py)


def _host_prep(z_e, w):
    """z_sq / e_sq via jax-CPU to mirror the reference's f32 reductions."""
    import jax
    import jax.numpy as jnp

    cpu = jax.devices("cpu")[0]
    with jax.default_device(cpu):
        zj = jnp.transpose(jnp.asarray(z_e), (0, 2, 1))
        z_sq = np.asarray(jnp.sum(zj * zj, axis=-1))          # (B, T) f32
        wj = jnp.asarray(w)
        e_sq = np.asarray(jnp.sum(wj * wj, axis=-1))          # (K,) f32
    return z_sq, e_sq


def _split_bf16(x):
    hi = x.astype(BF16)
    lo = (x - hi.astype(np.float32)).astype(BF16)
    return hi, lo


def _split_f16(x):
    hi = x.astype(np.float16)
    lo = (x - hi.astype(np.float32)).astype(np.float16)
    return hi, lo


def kernel(z_e, weight, _trace=False):
    z_e = np.ascontiguousarray(np.asarray(z_e, dtype=np.float32))
    w = np.ascontiguousarray(np.asarray(weight, dtype=np.float32))
    assert z_e.shape == (B, D, T) and w.shape == (K, D)

    z_sq, e_sq = _host_prep(z_e, w)

    # moving operands: column j holds code k = 511-j, scaled by -2 (exact)
    wt = np.ascontiguousarray((-2.0 * w.T)[:, ::-1]).astype(np.float32)  # (D, K)
    w_hi = wt.astype(np.float16)
    w_lob = (wt - w_hi.astype(np.float32)).astype(BF16)
    w_hi2 = np.ascontiguousarray(np.concatenate([w_hi, w_hi], axis=0))   # (128, K) f16
    w_lo2 = np.ascontiguousarray(np.concatenate([w_lob, w_lob], axis=0))  # (128, K) bf16
    esq_rev = np.ascontiguousarray(
        np.broadcast_to(e_sq[::-1], (TILE, K))
    ).astype(np.float32)

    in_maps = []
    for c in range(NCORES):
        zc = np.empty((D, NTOK), np.float32)
        for b in range(BPC):
            zc[:, b * T : (b + 1) * T] = z_e[BPC * c + b]
        zh16, zl16 = _split_f16(zc)
        z2_16 = np.empty((2 * D, NTOK), np.float16)
        z2_16[:D] = zh16
        z2_16[D:] = zl16
        zhb, zlb = _split_bf16(zc)
        z2_bf = np.empty((2 * D, NTOK), BF16)
        z2_bf[:D] = zhb
        z2_bf[D:] = zlb
        # zsq laid out [TILE, NTILES]: (p, j) -> token j*TILE + p
        zsqc = np.ascontiguousarray(
            z_sq[BPC * c : BPC * (c + 1)].reshape(NTILES, TILE).T
        ).astype(np.float32)
        in_maps.append(
            {
                "z2_16": z2_16,
                "z2_bf": z2_bf,
                "w_hi2": w_hi2,
                "w_lo2": w_lo2,
                "zsq_in": zsqc,
                "esq_rev": esq_rev,
            }
        )

    nc = _get_nc()
    global LAST_RESULTS
    # transient NRT/axon device hiccups have been observed; retry
    for attempt in range(3):
        try:
            LAST_RESULTS = run_bass_kernel_spmd(
                nc, in_maps, list(range(NCORES)), trace=_trace
            )
            break
        except Exception:
            if attempt == 2:
                raise
            import time as _time

            _time.sleep(2.0 * (attempt + 1))

    out = np.empty((B, T), np.int32)
    for c in range(NCORES):
        o = np.asarray(LAST_RESULTS.results[c]["idx_out"]).reshape(BPC, T)
        for b in range(BPC):
            out[BPC * c + b] = o[b]
    return out
